# revision 39
# baseline (speedup 1.0000x reference)
"""Causal segment-masked depthwise conv (K=5) + pointwise conv, 8-core SPMD.

Strategy (v3, bf16):
  Host: pack each batch row's segments into a global stream with 4 zeros
  before each segment (plain causal conv on the stream == per-segment
  left-zero-padded conv), split the stream evenly across 8 cores with a
  4-element halo, pre-transpose to [C, stream] bf16 and ship as ONE flat
  tensor per core; the device DMAs overlapping column-range pieces.
  Biases fold out of the device: out = Wpw.conv + (Wpw@b_dw + b_pw); the
  constant rides the ACT out-copies' per-partition bias operand.
  Device per core (stream 4160 = superblocks [512,1024,1024,1024,512] +
  64 tail; narrow edge superblocks shrink the startup latency and the
  final drain):
    dw conv:
      PE   : chunks 0,1 (diag-stationary bf16 matmuls, tap-major over
             512-blocks so each LDWEIGHTS serves the whole superblock;
             PSUM f32, one wide ACT copy -> bf16). diag built on DVE
             (chunks 0,1, needed first) and ACT (chunks 2,3, tail-only).
      DVE  : chunks 2,3 as tensor_scalar products + tensor_tensor adds,
             even-shift taps first (4x packed mode), odd-shift after
             (2x 2-port mode tolerates the misaligned read).
    pw: dch-pair major per superblock (pair 0,1 then 2,3) with j=2,3
        (DVE inputs) last; each weight load serves all blocks of the
        superblock; ACT copies PSUM -> bf16 adding the folded bias;
        per-pair stores spread over the three DMA queues, final stores
        split per-dch for a fast drain.
  The tail block (all 4 chunks on PE) is emitted mid-pipeline as filler
  for the first DVE-wait window.
  Host transposes back during gather and applies a sparse general-case
  correction for exotic segment overlap patterns (empty for contiguous
  partitions).
"""

import sys

sys.path.insert(0, "/opt/trn_rl_repo")

import numpy as np
import ml_dtypes

BF16 = ml_dtypes.bfloat16

B, L, C, K, S = 8, 4096, 512, 5, 8
NCORES = 8
CCH = C // 128                    # 4 channel chunks
WS = [512, 1024, 1024, 1024, 512]  # superblock widths
OFFS = [0, 512, 1536, 2560, 3584]  # superblock column offsets
NSB = len(WS)
SBW = sum(WS)                     # 4096
TAILW = 64
TOTW = SBW + TAILW                # 4160
XFW = 4 + TOTW + 12               # flat slab width (halo 4 + pad)

_cached = {}


def _build_nc():
    import concourse.mybir as mybir
    from concourse import bacc
    from concourse.tile import TileContext

    f32 = mybir.dt.float32
    bf16 = mybir.dt.bfloat16

    nc = bacc.Bacc(num_swdge_queues=1)
    xf_d = nc.declare_dram_parameter("xf", [128, CCH, XFW], bf16, isOutput=False)
    # cst: [0:20]=wdiag f32, [20:24]=bout f32
    cst_d = nc.declare_dram_parameter("cst", [128, CCH * K + CCH], f32, isOutput=False)
    identb_d = nc.declare_dram_parameter("identb", [128, 128], bf16, isOutput=False)
    wpwt_d = nc.declare_dram_parameter(
        "wpwt", [128, 2, CCH, 2, 128], bf16, isOutput=False
    )
    out_d = nc.declare_dram_parameter("out", [128, CCH, TOTW], bf16, isOutput=True)

    with TileContext(nc) as tc:
        with (
            tc.tile_pool(name="consts", bufs=1) as cpool,
            tc.tile_pool(name="xc0", bufs=3) as x0_pool,
            tc.tile_pool(name="xc1", bufs=2) as x1_pool,
            tc.tile_pool(name="xc2", bufs=3) as x2_pool,
            tc.tile_pool(name="xc3", bufs=3) as x3_pool,
            tc.tile_pool(name="acc2", bufs=2) as a2_pool,
            tc.tile_pool(name="acc3", bufs=2) as a3_pool,
            tc.tile_pool(name="tprod", bufs=2) as tp_pool,
            tc.tile_pool(name="dwt", bufs=4) as dwt_pool,
            tc.tile_pool(name="outsb", bufs=3) as ob_pool,
            tc.tile_pool(name="dps", bufs=2, space="PSUM") as dps,
            tc.tile_pool(name="pwps", bufs=2, space="PSUM") as pwps,
        ):
            xt = {}
            pools = {0: x0_pool, 1: x1_pool, 2: x2_pool, 3: x3_pool}

            def load(eng, sb, c, nch=1):
                w = WS[sb] + 8
                t = pools[c].tile(
                    [128, nch, w], bf16, tag=f"x{c}", name=f"x{c}_{sb}"
                )
                eng.dma_start(
                    out=t[:], in_=xf_d[:, c : c + nch, OFFS[sb] : OFFS[sb] + w]
                )
                for cc in range(c, c + nch):
                    xt[(sb, cc)] = (t, cc - c)
                return t

            def xsl(sb, c):
                t, i = xt[(sb, c)]
                return t[:, i, :]

            # ---- scalar ring: consts first (DVE diag + ACT bias need cst),
            # then pointwise weights in pair halves ----
            cst = cpool.tile([128, CCH * K + CCH], f32)
            nc.scalar.dma_start(out=cst[:], in_=cst_d[:])
            wdiag = cst[:, 0 : CCH * K]
            bout = cst[:, CCH * K : CCH * K + CCH]
            identb = cpool.tile([128, 128], bf16)
            nc.scalar.dma_start(out=identb[:], in_=identb_d[:])
            ident = identb[:]
            wpwt = cpool.tile([128, 2, CCH, 2, 128], bf16)
            nc.scalar.dma_start(out=wpwt[:, 0], in_=wpwt_d[:, 0])
            nc.scalar.dma_start(out=wpwt[:, 1], in_=wpwt_d[:, 1])

            # ---- sync ring: sb0 PE pieces first for the earliest conv ----
            load(nc.sync, 0, 0)
            load(nc.sync, 0, 1)
            xq = cpool.tile([128, CCH, TAILW + 8], bf16)
            nc.sync.dma_start(
                out=xq[:], in_=xf_d[:, :, SBW : SBW + TAILW + 8]
            )
            load(nc.sync, 1, 0, 2)

            # ---- gpsimd SWDGE ----
            warmz = cpool.tile([128, 512], bf16)
            nc.gpsimd.memset(warmz[:], 0.0)
            load(nc.gpsimd, 0, 2)
            load(nc.gpsimd, 0, 3)
            load(nc.gpsimd, 1, 2)
            load(nc.gpsimd, 1, 3)

            # remaining loads are emitted inside the superblock loop with a
            # two-superblock lookahead so no queue head blocks on pool slots
            def emit_loads(stage):
                if stage == 0:
                    load(nc.sync, 2, 0, 2)
                    load(nc.scalar, 2, 2, 2)
                elif stage == 1:
                    load(nc.sync, 3, 0, 2)
                    load(nc.scalar, 3, 2, 2)
                elif stage == 2:
                    load(nc.gpsimd, 4, 0, 2)
                    load(nc.gpsimd, 4, 2, 2)

            # PE warm-up while DMAs land
            for wi in range(6):
                wps = pwps.tile([128, 512], f32, tag="pwps", name=f"warm{wi}")
                nc.tensor.matmul(
                    wps[:], lhsT=warmz[:, 0:128], rhs=warmz[:], start=True, stop=True
                )

            # diag tiles: chunks 0,1 on DVE (needed by sb0 conv ASAP);
            # chunks 2,3 (tail-only) built on ACT later, see the sb loop
            diag = cpool.tile([128, CCH * K, 128], bf16)
            for u in range(2 * K):
                nc.vector.tensor_scalar_mul(diag[:, u, :], ident, wdiag[:, u : u + 1])

            # ---- DVE dw conv for one chunk over one superblock ----
            def dve_chunk(sb, c, pool):
                w = WS[sb]
                A = xsl(sb, c)
                acc = pool.tile([128, w], bf16, tag=f"a{c}", name=f"a{c}_{sb}")
                nc.vector.tensor_scalar_mul(
                    acc[:], A[:, 0:w], wdiag[:, c * K : c * K + 1]
                )
                for k in (2, 4, 1, 3):  # evens (aligned) first, odds after
                    tp = tp_pool.tile([128, w], bf16, tag="tp", name=f"tp{c}_{sb}_{k}")
                    nc.vector.tensor_scalar_mul(
                        tp[:], A[:, k : k + w], wdiag[:, c * K + k : c * K + k + 1]
                    )
                    nc.vector.tensor_add(acc[:], acc[:], tp[:])
                return acc

            # ---- PE dw conv for one chunk over one superblock (tap-major) ----
            def pe_conv(sb, c):
                w = WS[sb]
                nb = w // 512
                ps = dps.tile([128, w], f32, tag="dps", name=f"ps{c}_{sb}")
                x = xsl(sb, c)
                for k in range(K):
                    for bb in range(nb):
                        o = bb * 512
                        nc.tensor.matmul(
                            ps[:, o : o + 512],
                            lhsT=diag[:, c * K + k, :],
                            rhs=x[:, o + k : o + k + 512],
                            start=(k == 0),
                            stop=(k == K - 1),
                        )
                dt_ = dwt_pool.tile([128, w], bf16, tag="dwt", name=f"dwt{c}_{sb}")
                # late superblocks: DVE is idle by then, ACT is the bottleneck
                if sb >= 3:
                    nc.vector.tensor_copy(dt_[:], ps[:])
                else:
                    nc.scalar.copy(dt_[:], ps[:])
                return dt_

            # ---- pointwise for one superblock, one dch pair ----
            def pointwise_pair(sb, pair, rhs_of, stores):
                w = WS[sb]
                nb = w // 512
                pos = {}
                for dch in pair:
                    pos[dch] = pwps.tile([128, w], f32, tag="pwps", name=f"po{dch}_{sb}")
                pi = pair[0] // 2
                for jj in range(CCH):  # j=2,3 (DVE inputs) last
                    for i, dch in enumerate(pair):
                        for bb in range(nb):
                            o = bb * 512
                            nc.tensor.matmul(
                                pos[dch][:, o : o + 512],
                                lhsT=wpwt[:, pi, jj, i, :],
                                rhs=rhs_of[jj][:, o : o + 512],
                                start=(jj == 0),
                                stop=(jj == CCH - 1),
                            )
                ob = ob_pool.tile([128, 2, w], bf16, tag="ob", name=f"ob{pair[0]}_{sb}")
                for i, dch in enumerate(pair):
                    nc.scalar.add(ob[:, i, :], pos[dch][:], bout[:, dch : dch + 1])
                off = OFFS[sb]
                if len(stores) == 1:
                    stores[0].dma_start(
                        out=out_d[:, pair[0] : pair[0] + 2, off : off + w], in_=ob[:]
                    )
                else:  # split per dch across two queues (fast drain)
                    for i, dch in enumerate(pair):
                        stores[i].dma_start(
                            out=out_d[:, dch : dch + 1, off : off + w],
                            in_=ob[:, i : i + 1, :],
                        )

            # ---- tail block (cols 4096..4159), all 4 chunks on PE ----
            def tail_block():
                dwq = []
                for c in range(CCH):
                    ps = dps.tile([128, TAILW], f32, tag="dps", name=f"psq{c}")
                    for k in range(K):
                        nc.tensor.matmul(
                            ps[:],
                            lhsT=diag[:, c * K + k, :],
                            rhs=xq[:, c, k : k + TAILW],
                            start=(k == 0),
                            stop=(k == K - 1),
                        )
                    dt_ = dwt_pool.tile([128, TAILW], bf16, tag="dwt", name=f"dwq{c}")
                    nc.scalar.copy(dt_[:], ps[:])
                    dwq.append(dt_)
                pos = [
                    pwps.tile([128, TAILW], f32, tag="pwps", name=f"poq{dch}")
                    for dch in range(CCH)
                ]
                for j in range(CCH):
                    for dch in range(CCH):
                        nc.tensor.matmul(
                            pos[dch][:],
                            lhsT=wpwt[:, dch // 2, j, dch % 2, :],
                            rhs=dwq[j][:],
                            start=(j == 0),
                            stop=(j == CCH - 1),
                        )
                ob = ob_pool.tile([128, CCH, TAILW], bf16, tag="obq", name="ob_q")
                for dch in range(CCH):
                    nc.scalar.add(ob[:, dch, :], pos[dch][:], bout[:, dch : dch + 1])
                nc.gpsimd.dma_start(out=out_d[:, :, SBW : SBW + TAILW], in_=ob[:])

            # ---- main pipeline ----
            store_plan = {
                (0, 0): [nc.scalar],
                (0, 2): [nc.gpsimd],
                (1, 0): [nc.sync],
                (1, 2): [nc.gpsimd],
                (2, 0): [nc.gpsimd],
                (2, 2): [nc.scalar],
                (3, 0): [nc.sync],
                (3, 2): [nc.gpsimd],
                (4, 0): [nc.scalar],
                (4, 2): [nc.sync, nc.gpsimd],
            }
            for sb in range(NSB):
                emit_loads(sb)
                a2 = dve_chunk(sb, 2, a2_pool)
                dwt0 = pe_conv(sb, 0)
                dwt1 = pe_conv(sb, 1)
                a3 = dve_chunk(sb, 3, a3_pool)
                rhs_of = {0: dwt0, 1: dwt1, 2: a2, 3: a3}
                pointwise_pair(sb, (0, 1), rhs_of, store_plan[(sb, 0)])
                pointwise_pair(sb, (2, 3), rhs_of, store_plan[(sb, 2)])
                if sb == 0:
                    # tail-only diag tiles, built once ACT's sb0 copies are out
                    for u in range(2 * K, CCH * K):
                        nc.scalar.mul(diag[:, u, :], ident, wdiag[:, u : u + 1])
                    tail_block()

    nc.finalize()
    return nc


def _get_nc():
    if "nc" not in _cached:
        _cached["nc"] = _build_nc()
    return _cached["nc"]


def _analyze(segment_boundaries):
    starts = segment_boundaries[..., 0].astype(np.int64)  # [B,S]
    ends = segment_boundaries[..., 1].astype(np.int64)
    pos = np.arange(L)
    in_seg = (pos[None, None, :] >= starts[..., None]) & (
        pos[None, None, :] < ends[..., None]
    )  # [B,S,L]
    covered = in_seg.any(axis=1)
    seg_id = np.where(covered, in_seg.argmax(axis=1), -1)  # [B,L]
    return covered, seg_id


def kernel(x, segment_boundaries, w_dw, b_dw, w_pw, b_pw):
    from concourse.bass_utils import run_bass_kernel_spmd

    x = np.asarray(x, dtype=np.float32)
    sb = np.asarray(segment_boundaries)
    w_dw = np.asarray(w_dw, dtype=np.float32)
    b_dw = np.asarray(b_dw, dtype=np.float32)
    w_pw = np.asarray(w_pw, dtype=np.float32)
    b_pw = np.asarray(b_pw, dtype=np.float32)

    covered, seg_id = _analyze(sb)

    # ---- run decomposition + stream build ----
    pieces = []
    src_b_parts = []
    src_l_parts = []
    run_start_of = np.full((B, L), -1, np.int64)
    for b in range(B):
        sid = seg_id[b]
        change = np.nonzero(np.diff(sid) != 0)[0] + 1
        bounds = np.concatenate([[0], change, [L]])
        for s, e in zip(bounds[:-1], bounds[1:]):
            if sid[s] < 0:
                continue
            run_start_of[b, s:e] = s
            pieces.append(np.zeros((4, C), np.float32))
            src_b_parts.append(np.full(4, -1, np.int64))
            src_l_parts.append(np.full(4, -1, np.int64))
            pieces.append(x[b, s:e])
            src_b_parts.append(np.full(e - s, b, np.int64))
            src_l_parts.append(np.arange(s, e, dtype=np.int64))
    if pieces:
        stream = np.concatenate(pieces, axis=0)
        src_b = np.concatenate(src_b_parts)
        src_l = np.concatenate(src_l_parts)
    else:
        stream = np.zeros((0, C), np.float32)
        src_b = np.zeros(0, np.int64)
        src_l = np.zeros(0, np.int64)
    T = stream.shape[0]
    Q = -(-T // NCORES) if T else 1
    assert Q <= TOTW, f"stream quota {Q} too large"

    # ---- shared per-core inputs ----
    wdiag = np.ascontiguousarray(
        w_dw.reshape(CCH, 128, K).transpose(1, 0, 2).reshape(128, CCH * K)
    ).astype(np.float32)
    bias_out = w_pw @ b_dw + b_pw
    boutr = np.ascontiguousarray(bias_out.reshape(CCH, 128).T).astype(np.float32)
    cst = np.concatenate([wdiag, boutr], axis=1)
    identb = np.eye(128, dtype=np.float32).astype(BF16)
    wpwt4 = w_pw.reshape(CCH, 128, CCH, 128).transpose(3, 2, 0, 1)  # [c,jj,dch,d]
    wpwt = np.ascontiguousarray(
        wpwt4.transpose(0, 2, 1, 3)
        .reshape(128, 2, 2, CCH, 128)
        .transpose(0, 1, 3, 2, 4)
    ).astype(BF16)

    in_maps = []
    spans = []
    for i in range(NCORES):
        lo, hi = i * Q, min((i + 1) * Q, T)
        lo = min(lo, T)
        spans.append((lo, hi))
        buf = np.zeros((XFW, C), np.float32)
        if hi > lo:
            hlo = max(0, lo - 4)
            buf[4 - (lo - hlo) : 4 + (hi - lo)] = stream[hlo:hi]
        xf = np.ascontiguousarray(
            buf.T.reshape(CCH, 128, XFW).transpose(1, 0, 2)
        ).astype(BF16)
        in_maps.append({"xf": xf, "cst": cst, "identb": identb, "wpwt": wpwt})

    nc = _get_nc()
    res = run_bass_kernel_spmd(nc, in_maps, list(range(NCORES)))

    # ---- gather (device out is [128, CCH, TOTW] bf16) ----
    so_out = np.zeros((T, C), np.float32)
    for i, (lo, hi) in enumerate(spans):
        if hi > lo:
            # [p, ch, t] -> [t, ch*128+p]
            full = (
                np.asarray(res.results[i]["out"])
                .astype(np.float32)
                .transpose(2, 1, 0)
                .reshape(TOTW, C)
            )
            so_out[lo:hi] = full[: hi - lo]
    out = np.zeros((B, L, C), np.float32)
    mask = src_l >= 0
    out[src_b[mask], src_l[mask]] = so_out[mask]

    # ---- general-case sparse correction (pairwise mask vs run mask) ----
    need = []
    for d in range(1, K):
        m_ref = np.zeros((B, L), bool)
        m_ref[:, d:] = covered[:, d:] & (seg_id[:, d:] == seg_id[:, :-d])
        m_run = covered & (np.arange(L)[None, :] - run_start_of >= d)
        diff = m_ref.astype(np.int8) - m_run.astype(np.int8)
        if np.any(diff):
            bs, ls = np.nonzero(diff)
            need.append((d, bs, ls, diff[bs, ls].astype(np.float32)))
    if need:
        for d, bs, ls, sgn in need:
            xv_ = x[bs, ls - d, :]
            delta_dw = xv_ * w_dw[None, :, K - 1 - d] * sgn[:, None]
            out[bs, ls, :] += delta_dw @ w_pw.T

    return out


# revision 43
# speedup vs baseline: 1.1825x; 1.1825x over previous
"""Causal segment-masked depthwise conv (K=5) + pointwise conv, 8-core SPMD.

Strategy (v3, bf16):
  Host: pack each batch row's segments into a global stream with 4 zeros
  before each segment (plain causal conv on the stream == per-segment
  left-zero-padded conv), split the stream evenly across 8 cores with a
  4-element halo, pre-transpose to [C, stream] bf16 and ship as ONE flat
  tensor per core; the device DMAs overlapping column-range pieces.
  Biases fold out of the device: out = Wpw.conv + (Wpw@b_dw + b_pw); the
  constant rides the ACT out-copies' per-partition bias operand.
  Device per core (stream 4160 = superblocks [512,1024,1024,1024,512] +
  64 tail; narrow edge superblocks shrink the startup latency and the
  final drain):
    dw conv:
      PE   : chunks 0,1 (diag-stationary bf16 matmuls, tap-major over
             512-blocks so each LDWEIGHTS serves the whole superblock;
             PSUM f32, one wide ACT copy -> bf16). diag built on DVE
             (chunks 0,1, needed first) and ACT (chunks 2,3, tail-only).
      DVE  : chunks 2,3 as tensor_scalar products + tensor_tensor adds,
             even-shift taps first (4x packed mode), odd-shift after
             (2x 2-port mode tolerates the misaligned read).
    pw: dch-pair major per superblock (pair 0,1 then 2,3) with j=2,3
        (DVE inputs) last; each weight load serves all blocks of the
        superblock; ACT copies PSUM -> bf16 adding the folded bias;
        per-pair stores spread over the three DMA queues, final stores
        split per-dch for a fast drain.
  The tail block (all 4 chunks on PE) is emitted mid-pipeline as filler
  for the first DVE-wait window.
  Host transposes back during gather and applies a sparse general-case
  correction for exotic segment overlap patterns (empty for contiguous
  partitions).
"""

import sys

sys.path.insert(0, "/opt/trn_rl_repo")

import numpy as np
import ml_dtypes

BF16 = ml_dtypes.bfloat16

B, L, C, K, S = 8, 4096, 512, 5, 8
NCORES = 8
CCH = C // 128                    # 4 channel chunks
WS = [512, 1024, 1024, 1024, 512]  # superblock widths
OFFS = [0, 512, 1536, 2560, 3584]  # superblock column offsets
NSB = len(WS)
SBW = sum(WS)                     # 4096
TAILW = 64
TOTW = SBW + TAILW                # 4160
XFW = 4 + TOTW + 12               # flat slab width (halo 4 + pad)

_cached = {}


def _build_nc():
    import concourse.mybir as mybir
    from concourse import bacc
    from concourse.tile import TileContext

    f32 = mybir.dt.float32
    bf16 = mybir.dt.bfloat16

    nc = bacc.Bacc(num_swdge_queues=1)
    xf_d = nc.declare_dram_parameter("xf", [128, CCH, XFW], bf16, isOutput=False)
    # cst: [0:20]=wdiag f32, [20:24]=bout f32, [24:152]=identity f32
    cst_d = nc.declare_dram_parameter(
        "cst", [128, CCH * K + CCH + 128], f32, isOutput=False
    )
    wpwt_d = nc.declare_dram_parameter(
        "wpwt", [128, 2, CCH, 2, 128], bf16, isOutput=False
    )
    out_d = nc.declare_dram_parameter("out", [128, CCH, TOTW], bf16, isOutput=True)

    with TileContext(nc) as tc:
        with (
            tc.tile_pool(name="consts", bufs=1) as cpool,
            tc.tile_pool(name="xc0", bufs=3) as x0_pool,
            tc.tile_pool(name="xc1", bufs=2) as x1_pool,
            tc.tile_pool(name="xc2", bufs=3) as x2_pool,
            tc.tile_pool(name="xc3", bufs=3) as x3_pool,
            tc.tile_pool(name="acc2", bufs=2) as a2_pool,
            tc.tile_pool(name="acc3", bufs=2) as a3_pool,
            tc.tile_pool(name="tprod", bufs=2) as tp_pool,
            tc.tile_pool(name="dwt", bufs=4) as dwt_pool,
            tc.tile_pool(name="outsb", bufs=3) as ob_pool,
            tc.tile_pool(name="dps", bufs=2, space="PSUM") as dps,
            tc.tile_pool(name="pwps", bufs=2, space="PSUM") as pwps,
        ):
            xt = {}
            pools = {0: x0_pool, 1: x1_pool, 2: x2_pool, 3: x3_pool}

            def load(eng, sb, c, nch=1):
                w = WS[sb] + 8
                t = pools[c].tile(
                    [128, nch, w], bf16, tag=f"x{c}", name=f"x{c}_{sb}"
                )
                eng.dma_start(
                    out=t[:], in_=xf_d[:, c : c + nch, OFFS[sb] : OFFS[sb] + w]
                )
                for cc in range(c, c + nch):
                    xt[(sb, cc)] = (t, cc - c)
                return t

            def xsl(sb, c):
                t, i = xt[(sb, c)]
                return t[:, i, :]

            # ---- scalar ring: consts first (DVE diag + ACT bias need cst),
            # then pointwise weights in pair halves ----
            cst = cpool.tile([128, CCH * K + CCH + 128], f32)
            nc.scalar.dma_start(out=cst[:], in_=cst_d[:])
            wdiag = cst[:, 0 : CCH * K]
            bout = cst[:, CCH * K : CCH * K + CCH]
            ident = cst[:, CCH * K + CCH : CCH * K + CCH + 128]
            wpwt = cpool.tile([128, 2, CCH, 2, 128], bf16)
            nc.scalar.dma_start(out=wpwt[:, 0], in_=wpwt_d[:, 0])
            nc.scalar.dma_start(out=wpwt[:, 1], in_=wpwt_d[:, 1])

            # ---- sync ring: sb0 PE pieces first for the earliest conv ----
            load(nc.sync, 0, 0)
            load(nc.sync, 0, 1)
            xq = cpool.tile([128, CCH, TAILW + 8], bf16)
            nc.sync.dma_start(
                out=xq[:], in_=xf_d[:, :, SBW : SBW + TAILW + 8]
            )
            load(nc.sync, 1, 0, 2)

            # ---- gpsimd SWDGE ----
            warmz = cpool.tile([128, 512], bf16)
            nc.gpsimd.memset(warmz[:], 0.0)
            load(nc.gpsimd, 0, 2)
            load(nc.gpsimd, 0, 3)
            load(nc.gpsimd, 1, 2)
            load(nc.gpsimd, 1, 3)

            # remaining loads are emitted inside the superblock loop with a
            # two-superblock lookahead so no queue head blocks on pool slots
            def emit_loads(stage):
                if stage == 0:
                    load(nc.sync, 2, 0, 2)
                    load(nc.scalar, 2, 2, 2)
                elif stage == 1:
                    load(nc.sync, 3, 0, 2)
                    load(nc.scalar, 3, 2, 2)
                elif stage == 2:
                    load(nc.gpsimd, 4, 0, 2)
                    load(nc.gpsimd, 4, 2, 2)

            # PE warm-up while DMAs land
            for wi in range(6):
                wps = pwps.tile([128, 512], f32, tag="pwps", name=f"warm{wi}")
                nc.tensor.matmul(
                    wps[:], lhsT=warmz[:, 0:128], rhs=warmz[:], start=True, stop=True
                )

            # diag tiles: chunks 0,1 on DVE (needed by sb0 conv ASAP);
            # chunks 2,3 (tail-only) built on ACT later, see the sb loop
            diag = cpool.tile([128, CCH * K, 128], bf16)
            for u in range(2 * K):
                nc.vector.tensor_scalar_mul(diag[:, u, :], ident, wdiag[:, u : u + 1])

            # ---- DVE dw conv for one chunk over one superblock ----
            def dve_chunk(sb, c, pool):
                w = WS[sb]
                A = xsl(sb, c)
                acc = pool.tile([128, w], bf16, tag=f"a{c}", name=f"a{c}_{sb}")
                nc.vector.tensor_scalar_mul(
                    acc[:], A[:, 0:w], wdiag[:, c * K : c * K + 1]
                )
                for k in (2, 4, 1, 3):  # evens (aligned) first, odds after
                    tp = tp_pool.tile([128, w], bf16, tag="tp", name=f"tp{c}_{sb}_{k}")
                    nc.vector.tensor_scalar_mul(
                        tp[:], A[:, k : k + w], wdiag[:, c * K + k : c * K + k + 1]
                    )
                    nc.vector.tensor_add(acc[:], acc[:], tp[:])
                return acc

            # ---- PE dw conv for one chunk over one superblock (tap-major) ----
            def pe_conv(sb, c):
                w = WS[sb]
                nb = w // 512
                ps = dps.tile([128, w], f32, tag="dps", name=f"ps{c}_{sb}")
                x = xsl(sb, c)
                for k in range(K):
                    for bb in range(nb):
                        o = bb * 512
                        nc.tensor.matmul(
                            ps[:, o : o + 512],
                            lhsT=diag[:, c * K + k, :],
                            rhs=x[:, o + k : o + k + 512],
                            start=(k == 0),
                            stop=(k == K - 1),
                        )
                dt_ = dwt_pool.tile([128, w], bf16, tag="dwt", name=f"dwt{c}_{sb}")
                # late superblocks: DVE is idle by then, ACT is the bottleneck
                if sb >= 3:
                    nc.vector.tensor_copy(dt_[:], ps[:])
                else:
                    nc.scalar.copy(dt_[:], ps[:])
                return dt_

            # ---- pointwise for one superblock, one dch pair ----
            def pointwise_pair(sb, pair, rhs_of, stores):
                w = WS[sb]
                nb = w // 512
                pos = {}
                for dch in pair:
                    pos[dch] = pwps.tile([128, w], f32, tag="pwps", name=f"po{dch}_{sb}")
                pi = pair[0] // 2
                for jj in range(CCH):  # j=2,3 (DVE inputs) last
                    for i, dch in enumerate(pair):
                        for bb in range(nb):
                            o = bb * 512
                            nc.tensor.matmul(
                                pos[dch][:, o : o + 512],
                                lhsT=wpwt[:, pi, jj, i, :],
                                rhs=rhs_of[jj][:, o : o + 512],
                                start=(jj == 0),
                                stop=(jj == CCH - 1),
                            )
                ob = ob_pool.tile([128, 2, w], bf16, tag="ob", name=f"ob{pair[0]}_{sb}")
                for i, dch in enumerate(pair):
                    nc.scalar.add(ob[:, i, :], pos[dch][:], bout[:, dch : dch + 1])
                off = OFFS[sb]
                if len(stores) == 1:
                    stores[0].dma_start(
                        out=out_d[:, pair[0] : pair[0] + 2, off : off + w], in_=ob[:]
                    )
                else:  # split per dch across two queues (fast drain)
                    for i, dch in enumerate(pair):
                        stores[i].dma_start(
                            out=out_d[:, dch : dch + 1, off : off + w],
                            in_=ob[:, i : i + 1, :],
                        )

            # ---- tail block (cols 4096..4159), all 4 chunks on PE ----
            def tail_block():
                dwq = []
                for c in range(CCH):
                    ps = dps.tile([128, TAILW], f32, tag="dps", name=f"psq{c}")
                    for k in range(K):
                        nc.tensor.matmul(
                            ps[:],
                            lhsT=diag[:, c * K + k, :],
                            rhs=xq[:, c, k : k + TAILW],
                            start=(k == 0),
                            stop=(k == K - 1),
                        )
                    dt_ = dwt_pool.tile([128, TAILW], bf16, tag="dwt", name=f"dwq{c}")
                    nc.scalar.copy(dt_[:], ps[:])
                    dwq.append(dt_)
                pos = [
                    pwps.tile([128, TAILW], f32, tag="pwps", name=f"poq{dch}")
                    for dch in range(CCH)
                ]
                for j in range(CCH):
                    for dch in range(CCH):
                        nc.tensor.matmul(
                            pos[dch][:],
                            lhsT=wpwt[:, dch // 2, j, dch % 2, :],
                            rhs=dwq[j][:],
                            start=(j == 0),
                            stop=(j == CCH - 1),
                        )
                ob = ob_pool.tile([128, CCH, TAILW], bf16, tag="obq", name="ob_q")
                for dch in range(CCH):
                    nc.scalar.add(ob[:, dch, :], pos[dch][:], bout[:, dch : dch + 1])
                nc.gpsimd.dma_start(out=out_d[:, :, SBW : SBW + TAILW], in_=ob[:])

            # ---- main pipeline ----
            store_plan = {
                (0, 0): [nc.scalar],
                (0, 2): [nc.gpsimd],
                (1, 0): [nc.sync],
                (1, 2): [nc.gpsimd],
                (2, 0): [nc.gpsimd],
                (2, 2): [nc.scalar],
                (3, 0): [nc.sync],
                (3, 2): [nc.gpsimd],
                (4, 0): [nc.scalar],
                (4, 2): [nc.sync, nc.gpsimd],
            }
            for sb in range(NSB):
                emit_loads(sb)
                a2 = dve_chunk(sb, 2, a2_pool)
                dwt0 = pe_conv(sb, 0)
                dwt1 = pe_conv(sb, 1)
                a3 = dve_chunk(sb, 3, a3_pool)
                rhs_of = {0: dwt0, 1: dwt1, 2: a2, 3: a3}
                pointwise_pair(sb, (0, 1), rhs_of, store_plan[(sb, 0)])
                pointwise_pair(sb, (2, 3), rhs_of, store_plan[(sb, 2)])
                if sb == 0:
                    # tail-only diag tiles, built once ACT's sb0 copies are out
                    for u in range(2 * K, CCH * K):
                        nc.scalar.mul(diag[:, u, :], ident, wdiag[:, u : u + 1])
                    tail_block()

    nc.finalize()
    return nc


def _get_nc():
    if "nc" not in _cached:
        _cached["nc"] = _build_nc()
    return _cached["nc"]


def _analyze(segment_boundaries):
    starts = segment_boundaries[..., 0].astype(np.int64)  # [B,S]
    ends = segment_boundaries[..., 1].astype(np.int64)
    pos = np.arange(L)
    in_seg = (pos[None, None, :] >= starts[..., None]) & (
        pos[None, None, :] < ends[..., None]
    )  # [B,S,L]
    covered = in_seg.any(axis=1)
    seg_id = np.where(covered, in_seg.argmax(axis=1), -1)  # [B,L]
    return covered, seg_id


def kernel(x, segment_boundaries, w_dw, b_dw, w_pw, b_pw):
    from concourse.bass_utils import run_bass_kernel_spmd

    x = np.asarray(x, dtype=np.float32)
    sb = np.asarray(segment_boundaries)
    w_dw = np.asarray(w_dw, dtype=np.float32)
    b_dw = np.asarray(b_dw, dtype=np.float32)
    w_pw = np.asarray(w_pw, dtype=np.float32)
    b_pw = np.asarray(b_pw, dtype=np.float32)

    covered, seg_id = _analyze(sb)

    # ---- run decomposition + stream build ----
    pieces = []
    src_b_parts = []
    src_l_parts = []
    run_start_of = np.full((B, L), -1, np.int64)
    for b in range(B):
        sid = seg_id[b]
        change = np.nonzero(np.diff(sid) != 0)[0] + 1
        bounds = np.concatenate([[0], change, [L]])
        for s, e in zip(bounds[:-1], bounds[1:]):
            if sid[s] < 0:
                continue
            run_start_of[b, s:e] = s
            pieces.append(np.zeros((4, C), np.float32))
            src_b_parts.append(np.full(4, -1, np.int64))
            src_l_parts.append(np.full(4, -1, np.int64))
            pieces.append(x[b, s:e])
            src_b_parts.append(np.full(e - s, b, np.int64))
            src_l_parts.append(np.arange(s, e, dtype=np.int64))
    if pieces:
        stream = np.concatenate(pieces, axis=0)
        src_b = np.concatenate(src_b_parts)
        src_l = np.concatenate(src_l_parts)
    else:
        stream = np.zeros((0, C), np.float32)
        src_b = np.zeros(0, np.int64)
        src_l = np.zeros(0, np.int64)
    T = stream.shape[0]
    Q = -(-T // NCORES) if T else 1
    assert Q <= TOTW, f"stream quota {Q} too large"

    # ---- shared per-core inputs ----
    wdiag = np.ascontiguousarray(
        w_dw.reshape(CCH, 128, K).transpose(1, 0, 2).reshape(128, CCH * K)
    ).astype(np.float32)
    bias_out = w_pw @ b_dw + b_pw
    boutr = np.ascontiguousarray(bias_out.reshape(CCH, 128).T).astype(np.float32)
    cst = np.concatenate(
        [wdiag, boutr, np.eye(128, dtype=np.float32)], axis=1
    )
    wpwt4 = w_pw.reshape(CCH, 128, CCH, 128).transpose(3, 2, 0, 1)  # [c,jj,dch,d]
    wpwt = np.ascontiguousarray(
        wpwt4.transpose(0, 2, 1, 3)
        .reshape(128, 2, 2, CCH, 128)
        .transpose(0, 1, 3, 2, 4)
    ).astype(BF16)

    in_maps = []
    spans = []
    for i in range(NCORES):
        lo, hi = i * Q, min((i + 1) * Q, T)
        lo = min(lo, T)
        spans.append((lo, hi))
        buf = np.zeros((XFW, C), np.float32)
        if hi > lo:
            hlo = max(0, lo - 4)
            buf[4 - (lo - hlo) : 4 + (hi - lo)] = stream[hlo:hi]
        xf = np.ascontiguousarray(
            buf.T.reshape(CCH, 128, XFW).transpose(1, 0, 2)
        ).astype(BF16)
        in_maps.append({"xf": xf, "cst": cst, "wpwt": wpwt})

    nc = _get_nc()
    res = run_bass_kernel_spmd(nc, in_maps, list(range(NCORES)))

    # ---- gather (device out is [128, CCH, TOTW] bf16) ----
    so_out = np.zeros((T, C), np.float32)
    for i, (lo, hi) in enumerate(spans):
        if hi > lo:
            # [p, ch, t] -> [t, ch*128+p]
            full = (
                np.asarray(res.results[i]["out"])
                .astype(np.float32)
                .transpose(2, 1, 0)
                .reshape(TOTW, C)
            )
            so_out[lo:hi] = full[: hi - lo]
    out = np.zeros((B, L, C), np.float32)
    mask = src_l >= 0
    out[src_b[mask], src_l[mask]] = so_out[mask]

    # ---- general-case sparse correction (pairwise mask vs run mask) ----
    need = []
    for d in range(1, K):
        m_ref = np.zeros((B, L), bool)
        m_ref[:, d:] = covered[:, d:] & (seg_id[:, d:] == seg_id[:, :-d])
        m_run = covered & (np.arange(L)[None, :] - run_start_of >= d)
        diff = m_ref.astype(np.int8) - m_run.astype(np.int8)
        if np.any(diff):
            bs, ls = np.nonzero(diff)
            need.append((d, bs, ls, diff[bs, ls].astype(np.float32)))
    if need:
        for d, bs, ls, sgn in need:
            xv_ = x[bs, ls - d, :]
            delta_dw = xv_ * w_dw[None, :, K - 1 - d] * sgn[:, None]
            out[bs, ls, :] += delta_dw @ w_pw.T

    return out
